# revision 3
# baseline (speedup 1.0000x reference)
"""Trainium2 Bass kernel for nn_DivergenceRN (gnn_message_passing).

Reference computes, per batch b:
    Z_XX[b,i,:] = max_j relu(X[b,j]@W1a_xx + X[b,i]@W1c_xx + b1_xx) @ W_xx2
    Z_YX[b,i,:] = max_j relu(Y[b,j]@W1a_yx + X[b,i]@W1c_yx + b1_yx) @ W_yx2
    Z = sum_i (Z_XX - Z_YX);  out = relu(cat(Z,Z)@Wd1+bd1)@Wd2+bd2
(The YY / XY branches in the reference are dead code — output-independent.)

Device layout: partitions = 64 h-channels x {xx, yx} = 128; free dim = j.

Key structure (v3): pre[i,j,h] = pa[j,h] + pc[i,h] + b1[h], and pa is
i-independent — so pa is computed ONCE per batch into a PSUM bank
(4 fp32r matmuls total) instead of being re-derived per i with two PE
passes like the old per-i pipeline.  Per i the work is then:
  1. rp = relu(pa + pc[:,i]) -> bf16 SBUF.  The per-i column add is a
     per-partition scalar, so it can ride the Act engine's bias port
     (pa from PSUM), a DVE tensor_scalar (4x mode on a bf16 SBUF copy
     of pa), or a GpSimd tensor_scalar — split by RELU_POLICY so no
     single engine becomes the bottleneck.
  2. one weight-stationary bf16 matmul vs blockdiag(W_xx2, W_yx2),
     two i's per 2-bank PSUM tile.
  3. max over j: Act copies h[:,192:384] PSUM->SBUF bf16 (batched over
     the 2-i tile), then ONE DVE tensor_tensor_scan with op0=op1=max
     runs state = max(state, h_psum[t], h_sbuf[t]) — 2 elems/cycle,
     twice tensor_reduce's rate.  The scan's last output column is the
     max; a tiny strided copy extracts it for 8 i's at once.
Sharding: i in [0,384) split across 8 cores (48 rows per core per batch).
Host does the final cross-core sum, b2/decoder folding (tiny).
"""

import numpy as np

import concourse.bacc as bacc
import concourse.mybir as mybir
import concourse.tile as tile
from concourse.bass_utils import run_bass_kernel_spmd

B, N, M, D, H = 4, 384, 384, 64, 64
NCORES = 8
NI = N // NCORES          # i-rows per core per batch
P = 2 * H                 # 128 partitions: h x {xx, yx}
HALF = N // 2
GEXT = 8                  # i's per max-column extract
BLOB_W = B * N + B * NI + 2 * P   # packed input blob columns

F32 = mybir.dt.float32
F32R = mybir.dt.float32r
BF16 = mybir.dt.bfloat16
AX = mybir.AxisListType
ALU = mybir.AluOpType
ACTF = mybir.ActivationFunctionType

# Per-i relu engine split (fractions of i's): DVE / Act / GpSimd.
RELU_FRACS = {"D": 0.35, "A": 0.25, "G": 0.40}


def _relu_policy(n):
    """Bresenham-spread the relu engine assignment over n i's."""
    debt = dict.fromkeys(RELU_FRACS, 0.0)
    seq = []
    for _ in range(n):
        for e, f in RELU_FRACS.items():
            debt[e] += f
        pick = max(debt, key=debt.get)
        debt[pick] -= 1.0
        seq.append(pick)
    return seq


def build_nc():
    nc = bacc.Bacc("TRN2", target_bir_lowering=False)

    blob = nc.dram_tensor("blob", [P, BLOB_W], F32R, kind="ExternalInput")
    w2bd16 = nc.dram_tensor("w2bd16", [P, P], BF16, kind="ExternalInput")
    out = nc.dram_tensor("out", [P, B], F32, kind="ExternalOutput")

    policy = _relu_policy(B * NI)

    with tile.TileContext(nc) as tc:
        with (
            tc.tile_pool(name="singles", bufs=1) as singles,
            tc.tile_pool(name="rp", bufs=6) as rp_pool,
            tc.tile_pool(name="hcp", bufs=3) as hc_pool,
            tc.tile_pool(name="scr", bufs=2) as scr_pool,
            tc.tile_pool(name="pap", bufs=1, space="PSUM") as pa_pool,
            tc.tile_pool(name="hps", bufs=2, space="PSUM") as h_pool,
        ):
            blob_s = singles.tile([P, BLOB_W], F32R)
            w2bd_s16 = singles.tile([P, P], BF16)
            pa16 = singles.tile([P, B, N], BF16)
            pc_s = singles.tile([P, B * NI], F32)
            strip = singles.tile([P, B, NI], F32)
            acc = singles.tile([P, B], F32)

            # Preload the Relu activation table while the DMA streams in.
            warm = singles.tile([P, 1], F32)
            nc.vector.memset(warm, 0.0)
            nc.scalar.activation(out=warm, in_=warm, func=ACTF.Relu, scale=1.0)

            nc.sync.dma_start(out=blob_s[:, :], in_=blob[:, :])
            nc.sync.dma_start(out=w2bd_s16, in_=w2bd16[:, :])

            o = 0
            xyt_s = blob_s[:, o : o + B * N].rearrange("p (b n) -> p b n", b=B)
            o += B * N
            # xitb: [65, B*NI] — rows 0-63 Xi^T, row 64 = 1.0 (bias lane)
            xitb_s = blob_s[0 : D + 1, o : o + B * NI]
            o += B * NI
            w1ad_s = blob_s[:, o : o + P]
            o += P
            # w1cb: [65, 128] — rows 0-63 = [W1c_xx | W1c_yx], row 64 = b1^T
            w1cb_s = blob_s[0 : D + 1, o : o + P]
            o += P
            assert o == BLOB_W

            # --- setup: pa[b] = blockdiag(W1a)^T @ [X^T;Y^T][b], kept in
            # PSUM for the Act-relu path and copied to bf16 SBUF for the
            # DVE/GpSimd-relu paths.  pc = [W1c|b1]^T @ [Xi;1] for all (b,i).
            pa_ps = []
            for b in range(B):
                pa_b = pa_pool.tile([P, 512], F32, tag=f"pa{b}")
                nc.tensor.matmul(
                    pa_b[:, 0:N], lhsT=w1ad_s, rhs=xyt_s[:, b, :],
                    start=True, stop=True,
                )
                pa_ps.append(pa_b)

            pc_ps = h_pool.tile([P, 2, 512], F32, tag="h")
            nc.tensor.matmul(
                pc_ps[:, 0, 0 : B * NI], lhsT=w1cb_s, rhs=xitb_s,
                start=True, stop=True,
            )
            nc.scalar.activation(
                out=pc_s, in_=pc_ps[:, 0, 0 : B * NI], func=ACTF.Copy, scale=1.0
            )
            for b in range(B):
                nc.scalar.activation(
                    out=pa16[:, b, :], in_=pa_ps[b][:, 0:N],
                    func=ACTF.Copy, scale=1.0,
                )

            # --- main loop (2 i's per PSUM tile, GEXT i's per extract) ---
            for b in range(B):
                for i0 in range(0, NI, GEXT):
                    scr = scr_pool.tile([P, GEXT, HALF], F32, tag="scr")
                    for g0 in range(0, GEXT, 2):
                        h_ps = h_pool.tile([P, 2, 512], F32, tag="h")
                        hcp = hc_pool.tile([P, 2, HALF], BF16, tag="hc")
                        for g in range(2):
                            ii = i0 + g0 + g
                            k = b * NI + ii
                            rp = rp_pool.tile([P, N], BF16, tag="rp")
                            bias_ap = pc_s[:, k : k + 1]
                            eng = policy[k]
                            if eng == "A":
                                nc.scalar.activation(
                                    out=rp, in_=pa_ps[b][:, 0:N],
                                    func=ACTF.Relu, bias=bias_ap, scale=1.0,
                                )
                            elif eng == "G":
                                nc.gpsimd.tensor_scalar(
                                    out=rp, in0=pa16[:, b, :],
                                    scalar1=bias_ap, scalar2=0.0,
                                    op0=ALU.add, op1=ALU.max,
                                )
                            else:
                                nc.vector.tensor_scalar(
                                    out=rp, in0=pa16[:, b, :],
                                    scalar1=bias_ap, scalar2=0.0,
                                    op0=ALU.add, op1=ALU.max,
                                )
                            nc.tensor.matmul(
                                h_ps[:, g, 0:N], lhsT=w2bd_s16, rhs=rp,
                                start=True, stop=True,
                            )
                        # batched PSUM->SBUF bf16 copy of the second halves
                        nc.scalar.activation(
                            out=hcp, in_=h_ps[:, :, HALF:N],
                            func=ACTF.Copy, scale=1.0,
                        )
                        for g in range(2):
                            nc.vector.tensor_tensor_scan(
                                out=scr[:, g0 + g, :],
                                data0=h_ps[:, g, 0:HALF],
                                data1=hcp[:, g, :],
                                initial=-1e30,
                                op0=ALU.max, op1=ALU.max,
                            )
                    # strided extract of the scans' final columns
                    nc.vector.tensor_copy(
                        out=strip[:, b, i0 : i0 + GEXT],
                        in_=scr[:, :, HALF - 1],
                    )

            nc.vector.tensor_reduce(
                out=acc, in_=strip, axis=AX.X, op=ALU.add
            )
            nc.sync.dma_start(out=out[:, :], in_=acc[:, :])

    nc.compile()
    return nc


def _prep_inputs(X, Y, W_xx1, W_yx1, b_xx1, b_yx1, W_xx2, W_yx2):
    """Host-side input prep shared by all cores (except xit)."""
    f = np.float32
    XYT = np.ascontiguousarray(
        np.concatenate([X.transpose(0, 2, 1), Y.transpose(0, 2, 1)], axis=1), f
    )  # [B, 128, N]
    W1ad = np.zeros((P, P), f)
    W1ad[:D, :H] = W_xx1[:D]
    W1ad[D:, H:] = W_yx1[:D]
    W1c = np.ascontiguousarray(np.concatenate([W_xx1[D:], W_yx1[D:]], axis=1), f)
    b1v = np.concatenate([b_xx1, b_yx1]).reshape(P, 1).astype(f)
    W2bd = np.zeros((P, P), f)
    W2bd[:H, :H] = W_xx2
    W2bd[H:, H:] = W_yx2
    return XYT, W1ad, W1c, b1v, W2bd


def _pack_blob(XYT, XiT, W1ad, W1c, b1v, W2bd):
    """Pack per-core inputs into the [P, BLOB_W] blob (see build_nc)."""
    f = np.float32
    blob = np.zeros((P, BLOB_W), f)
    o = 0
    blob[:, o : o + B * N] = XYT.transpose(1, 0, 2).reshape(P, B * N)
    o += B * N
    blob[:D, o : o + B * NI] = XiT.transpose(1, 0, 2).reshape(D, B * NI)
    blob[D, o : o + B * NI] = 1.0
    o += B * NI
    blob[:, o : o + P] = W1ad
    o += P
    blob[:D, o : o + P] = W1c
    blob[D, o : o + P] = b1v[:, 0]
    o += P
    assert o == BLOB_W
    return blob


def kernel(
    X, Y,
    W_xx1, b_xx1, W_xx2, b_xx2,
    W_xy1, b_xy1, W_xy2, b_xy2,
    W_yx1, b_yx1, W_yx2, b_yx2,
    W_yy1, b_yy1, W_yy2, b_yy2,
    Wd1, bd1, Wd2, bd2,
    _trace=False, _tmpdir=None,
):
    f = np.float32
    X = np.asarray(X, f)
    Y = np.asarray(Y, f)
    XYT, W1ad, W1c, b1v, W2bd = _prep_inputs(
        X, Y, W_xx1, W_yx1, b_xx1, b_yx1, W_xx2, W_yx2
    )
    import ml_dtypes
    W2bd16 = np.ascontiguousarray(W2bd.astype(ml_dtypes.bfloat16))

    in_maps = []
    for c in range(NCORES):
        XiT = np.ascontiguousarray(
            X[:, c * NI : (c + 1) * NI, :].transpose(0, 2, 1), f
        )  # [B, 64, NI]
        in_maps.append(
            {
                "blob": _pack_blob(XYT, XiT, W1ad, W1c, b1v, W2bd),
                "w2bd16": W2bd16,
            }
        )

    nc = build_nc()
    res = run_bass_kernel_spmd(
        nc,
        in_maps,
        core_ids=list(range(NCORES)),
        trace=_trace,
        tmpdir=_tmpdir,
    )
    acc = np.zeros((P, B), np.float64)
    for r in res.results:
        acc += r["out"].astype(np.float64)
    acc = acc.astype(f)

    # acc[k, b] = sum_i max_j (relu_pre @ W2)[k]  for xx (k<64) / yx (k>=64)
    Zdiff = (acc[:H] - acc[H:]).T + N * (b_xx2 - b_yx2)[None, :]  # [B, H]
    z = np.concatenate([Zdiff, Zdiff], axis=1).astype(f)  # [B, 2H]
    h = np.maximum(z @ Wd1 + bd1, 0.0).astype(f)
    outv = (h @ Wd2 + bd2).astype(f)
    if _trace:
        return outv, res
    return outv


# revision 4
# speedup vs baseline: 3.9415x; 3.9415x over previous
"""Trainium2 Bass kernel for nn_DivergenceRN (gnn_message_passing).

Reference computes, per batch b:
    Z_XX[b,i,:] = max_j relu(X[b,j]@W1a_xx + X[b,i]@W1c_xx + b1_xx) @ W_xx2
    Z_YX[b,i,:] = max_j relu(Y[b,j]@W1a_yx + X[b,i]@W1c_yx + b1_yx) @ W_yx2
    Z = sum_i (Z_XX - Z_YX);  out = relu(cat(Z,Z)@Wd1+bd1)@Wd2+bd2
(The YY / XY branches in the reference are dead code — output-independent.)

Device layout: partitions = 64 h-channels x {xx, yx} = 128; free dim = j.

v4: the three per-i engine legs are balanced against HW-measured op
costs (PE matmul+ldw ~220ns/pass, Act batched bias-free relu ~450ns/i,
Act bias-ptr relu ~580ns/i, DVE pair-batched psum max-reduce ~480ns/i).
The baseline is PE-bound (3 matmul passes per i: pa-recompute, per-i
bias broadcast, W2).  Here a fraction of i's instead uses the Act
engine's bias port — relu(pa16 + pc[:,i]) straight from a bf16 SBUF
copy of pa — skipping both pre matmuls for those i's, which moves work
from the PE (the bottleneck) to Act's headroom.  PSUM banks live as
4 x [128,2,512] work tiles with a single lifecycle per i-pair:
pre matmuls -> batched relu -> W2 matmul overwrites -> pair-batched
max-reduce writes strip directly (no extract step).
Sharding: i in [0,384) split across 8 cores (48 rows per core per batch).
Host does the final cross-core sum, b2/decoder folding (tiny).
"""

import numpy as np

import concourse.bacc as bacc
import concourse.mybir as mybir
import concourse.tile as tile
from concourse.bass_utils import run_bass_kernel_spmd

B, N, M, D, H = 4, 384, 384, 64, 64
NCORES = 8
NI = N // NCORES          # i-rows per core per batch
P = 2 * H                 # 128 partitions: h x {xx, yx}
BLOB_W = B * N + B * NI + 2 * P   # packed input blob columns

F32 = mybir.dt.float32
F32R = mybir.dt.float32r
BF16 = mybir.dt.bfloat16
AX = mybir.AxisListType
ALU = mybir.AluOpType
ACTF = mybir.ActivationFunctionType

# Which i-pairs take the Act-bias path (no PE pre-matmuls). Pattern is
# tiled over the pair index; 2/5 => ~40% of pairs.
ACT_BIAS_PAIRS = (1, 3)
ACT_BIAS_MOD = 5


def build_nc():
    nc = bacc.Bacc("TRN2", target_bir_lowering=False)

    blob = nc.dram_tensor("blob", [P, BLOB_W], F32R, kind="ExternalInput")
    w2bd16 = nc.dram_tensor("w2bd16", [P, P], BF16, kind="ExternalInput")
    out = nc.dram_tensor("out", [P, B], F32, kind="ExternalOutput")

    with tile.TileContext(nc) as tc:
        with (
            tc.tile_pool(name="singles", bufs=1) as singles,
            tc.tile_pool(name="rp", bufs=3) as rp_pool,
            tc.tile_pool(name="wk", bufs=4, space="PSUM") as wk_pool,
        ):
            blob_s = singles.tile([P, BLOB_W], F32R)
            w2bd_s16 = singles.tile([P, P], BF16)
            pa16 = singles.tile([P, B, N], BF16)
            pc_s = singles.tile([P, B * NI], F32)
            strip = singles.tile([P, B, NI], F32)
            acc = singles.tile([P, B], F32)

            # Preload the Relu activation table while the DMA streams in.
            warm = singles.tile([P, 1], F32)
            nc.vector.memset(warm, 0.0)
            nc.scalar.activation(out=warm, in_=warm, func=ACTF.Relu, scale=1.0)

            nc.sync.dma_start(out=blob_s[:, :], in_=blob[:, :])
            nc.sync.dma_start(out=w2bd_s16, in_=w2bd16[:, :])

            o = 0
            xyt_s = blob_s[:, o : o + B * N].rearrange("p (b n) -> p b n", b=B)
            o += B * N
            # xitb: [65, B*NI] — rows 0-63 Xi^T, row 64 = 1.0 (bias lane)
            xitb_s = blob_s[0 : D + 1, o : o + B * NI]
            o += B * NI
            w1ad_s = blob_s[:, o : o + P]
            o += P
            # w1cb: [65, 128] — rows 0-63 = [W1c_xx | W1c_yx], row 64 = b1^T
            w1cb_s = blob_s[0 : D + 1, o : o + P]
            o += P
            assert o == BLOB_W

            # --- setup: pc = [W1c|b1]^T @ [Xi;1] (includes b1) for all
            # (b,i); pa16[b] = bf16(blockdiag(W1a)^T @ [X^T;Y^T][b]) for
            # the Act-bias path.
            pc_ps = wk_pool.tile([P, 2, 512], F32, tag="wk")
            nc.tensor.matmul(
                pc_ps[:, 0, 0 : B * NI], lhsT=w1cb_s, rhs=xitb_s,
                start=True, stop=True,
            )
            nc.scalar.activation(
                out=pc_s, in_=pc_ps[:, 0, 0 : B * NI], func=ACTF.Copy, scale=1.0
            )
            for b0 in range(0, B, 2):
                pa_ps = wk_pool.tile([P, 2, 512], F32, tag="wk")
                for g in range(2):
                    nc.tensor.matmul(
                        pa_ps[:, g, 0:N], lhsT=w1ad_s, rhs=xyt_s[:, b0 + g, :],
                        start=True, stop=True,
                    )
                    nc.vector.tensor_copy(
                        out=pa16[:, b0 + g, :], in_=pa_ps[:, g, 0:N]
                    )

            # --- main loop: 2 i's per work tile ---
            for b in range(B):
                for pair in range(NI // 2):
                    ii = 2 * pair
                    k0 = b * NI + ii
                    wk = wk_pool.tile([P, 2, 512], F32, tag="wk")
                    rp2 = rp_pool.tile([P, 2, N], BF16, tag="rp")
                    act_bias = (pair % ACT_BIAS_MOD) in ACT_BIAS_PAIRS
                    if act_bias:
                        # relu(pa16 + pc[:,k]) on the Act bias port; the
                        # PE only runs the W2 matmul for these i's.
                        for g in range(2):
                            nc.scalar.activation(
                                out=rp2[:, g, :], in_=pa16[:, b, :],
                                func=ACTF.Relu,
                                bias=pc_s[:, k0 + g : k0 + g + 1], scale=1.0,
                            )
                    else:
                        # pre = pa (recompute) + pc-broadcast, then one
                        # batched bias-free relu over the pair.
                        for g in range(2):
                            nc.tensor.matmul(
                                wk[:, g, 0:N], lhsT=w1ad_s, rhs=xyt_s[:, b, :],
                                start=True, stop=False,
                            )
                            nc.tensor.matmul(
                                wk[:, g, 0:N], lhsT=w1cb_s,
                                rhs=xitb_s[:, k0 + g : k0 + g + 1].broadcast_to(
                                    [D + 1, N]
                                ),
                                start=False, stop=True,
                            )
                        nc.scalar.activation(
                            out=rp2, in_=wk[:, :, 0:N],
                            func=ACTF.Relu, scale=1.0,
                        )
                    for g in range(2):
                        nc.tensor.matmul(
                            wk[:, g, 0:N], lhsT=w2bd_s16, rhs=rp2[:, g, :],
                            start=True, stop=True,
                        )
                    nc.vector.tensor_reduce(
                        out=strip[:, b, ii : ii + 2], in_=wk[:, :, 0:N],
                        axis=AX.X, op=ALU.max,
                    )

            nc.vector.tensor_reduce(
                out=acc, in_=strip, axis=AX.X, op=ALU.add
            )
            nc.sync.dma_start(out=out[:, :], in_=acc[:, :])

    nc.compile()
    return nc


def _prep_inputs(X, Y, W_xx1, W_yx1, b_xx1, b_yx1, W_xx2, W_yx2):
    """Host-side input prep shared by all cores (except xit)."""
    f = np.float32
    XYT = np.ascontiguousarray(
        np.concatenate([X.transpose(0, 2, 1), Y.transpose(0, 2, 1)], axis=1), f
    )  # [B, 128, N]
    W1ad = np.zeros((P, P), f)
    W1ad[:D, :H] = W_xx1[:D]
    W1ad[D:, H:] = W_yx1[:D]
    W1c = np.ascontiguousarray(np.concatenate([W_xx1[D:], W_yx1[D:]], axis=1), f)
    b1v = np.concatenate([b_xx1, b_yx1]).reshape(P, 1).astype(f)
    W2bd = np.zeros((P, P), f)
    W2bd[:H, :H] = W_xx2
    W2bd[H:, H:] = W_yx2
    return XYT, W1ad, W1c, b1v, W2bd


def _pack_blob(XYT, XiT, W1ad, W1c, b1v, W2bd):
    """Pack per-core inputs into the [P, BLOB_W] blob (see build_nc)."""
    f = np.float32
    blob = np.zeros((P, BLOB_W), f)
    o = 0
    blob[:, o : o + B * N] = XYT.transpose(1, 0, 2).reshape(P, B * N)
    o += B * N
    blob[:D, o : o + B * NI] = XiT.transpose(1, 0, 2).reshape(D, B * NI)
    blob[D, o : o + B * NI] = 1.0
    o += B * NI
    blob[:, o : o + P] = W1ad
    o += P
    blob[:D, o : o + P] = W1c
    blob[D, o : o + P] = b1v[:, 0]
    o += P
    assert o == BLOB_W
    return blob


def kernel(
    X, Y,
    W_xx1, b_xx1, W_xx2, b_xx2,
    W_xy1, b_xy1, W_xy2, b_xy2,
    W_yx1, b_yx1, W_yx2, b_yx2,
    W_yy1, b_yy1, W_yy2, b_yy2,
    Wd1, bd1, Wd2, bd2,
    _trace=False, _tmpdir=None,
):
    f = np.float32
    X = np.asarray(X, f)
    Y = np.asarray(Y, f)
    XYT, W1ad, W1c, b1v, W2bd = _prep_inputs(
        X, Y, W_xx1, W_yx1, b_xx1, b_yx1, W_xx2, W_yx2
    )
    import ml_dtypes
    W2bd16 = np.ascontiguousarray(W2bd.astype(ml_dtypes.bfloat16))

    in_maps = []
    for c in range(NCORES):
        XiT = np.ascontiguousarray(
            X[:, c * NI : (c + 1) * NI, :].transpose(0, 2, 1), f
        )  # [B, 64, NI]
        in_maps.append(
            {
                "blob": _pack_blob(XYT, XiT, W1ad, W1c, b1v, W2bd),
                "w2bd16": W2bd16,
            }
        )

    nc = build_nc()
    res = run_bass_kernel_spmd(
        nc,
        in_maps,
        core_ids=list(range(NCORES)),
        trace=_trace,
        tmpdir=_tmpdir,
    )
    acc = np.zeros((P, B), np.float64)
    for r in res.results:
        acc += r["out"].astype(np.float64)
    acc = acc.astype(f)

    # acc[k, b] = sum_i max_j (relu_pre @ W2)[k]  for xx (k<64) / yx (k>=64)
    Zdiff = (acc[:H] - acc[H:]).T + N * (b_xx2 - b_yx2)[None, :]  # [B, H]
    z = np.concatenate([Zdiff, Zdiff], axis=1).astype(f)  # [B, 2H]
    h = np.maximum(z @ Wd1 + bd1, 0.0).astype(f)
    outv = (h @ Wd2 + bd2).astype(f)
    if _trace:
        return outv, res
    return outv


# revision 7
# speedup vs baseline: 4.3450x; 1.1024x over previous
"""Trainium2 Bass kernel for nn_DivergenceRN (gnn_message_passing).

Reference computes, per batch b:
    Z_XX[b,i,:] = max_j relu(X[b,j]@W1a_xx + X[b,i]@W1c_xx + b1_xx) @ W_xx2
    Z_YX[b,i,:] = max_j relu(Y[b,j]@W1a_yx + X[b,i]@W1c_yx + b1_yx) @ W_yx2
    Z = sum_i (Z_XX - Z_YX);  out = relu(cat(Z,Z)@Wd1+bd1)@Wd2+bd2
(The YY / XY branches in the reference are dead code — output-independent.)

Device layout: partitions = 64 h-channels x {xx, yx} = 128; free dim = j.

v4: the three per-i engine legs are balanced against HW-measured op
costs (PE matmul+ldw ~220ns/pass, Act batched bias-free relu ~450ns/i,
Act bias-ptr relu ~580ns/i, DVE pair-batched psum max-reduce ~480ns/i).
The baseline is PE-bound (3 matmul passes per i: pa-recompute, per-i
bias broadcast, W2).  Here a fraction of i's instead uses the Act
engine's bias port — relu(pa16 + pc[:,i]) straight from a bf16 SBUF
copy of pa — skipping both pre matmuls for those i's, which moves work
from the PE (the bottleneck) to Act's headroom.  PSUM banks live as
4 x [128,2,512] work tiles with a single lifecycle per i-pair:
pre matmuls -> batched relu -> W2 matmul overwrites -> pair-batched
max-reduce writes strip directly (no extract step).
Sharding: i in [0,384) split across 8 cores (48 rows per core per batch).
Host does the final cross-core sum, b2/decoder folding (tiny).
"""

import numpy as np

import concourse.bacc as bacc
import concourse.mybir as mybir
import concourse.tile as tile
from concourse.bass_utils import run_bass_kernel_spmd

B, N, M, D, H = 4, 384, 384, 64, 64
NCORES = 8
NI = N // NCORES          # i-rows per core per batch
P = 2 * H                 # 128 partitions: h x {xx, yx}
BLOB_W = B * N + B * NI + 2 * P   # packed input blob columns

F32 = mybir.dt.float32
F32R = mybir.dt.float32r
BF16 = mybir.dt.bfloat16
AX = mybir.AxisListType
ALU = mybir.AluOpType
ACTF = mybir.ActivationFunctionType

# Which i-pairs take the Act-bias path (no PE pre-matmuls). Pattern is
# tiled over the pair index; 2/5 => ~40% of pairs.
ACT_BIAS_PAIRS = (1, 3)
ACT_BIAS_MOD = 5


def build_nc():
    nc = bacc.Bacc("TRN2", target_bir_lowering=False)

    blob = nc.dram_tensor("blob", [P, BLOB_W], F32R, kind="ExternalInput")
    w2bd16 = nc.dram_tensor("w2bd16", [P, P], BF16, kind="ExternalInput")
    out = nc.dram_tensor("out", [P, B], F32, kind="ExternalOutput")

    with tile.TileContext(nc) as tc:
        with (
            tc.tile_pool(name="singles", bufs=1) as singles,
            tc.tile_pool(name="rp", bufs=4) as rp_pool,
            tc.tile_pool(name="wk", bufs=4, space="PSUM") as wk_pool,
        ):
            blob_s = singles.tile([P, BLOB_W], F32R)
            w2bd_s16 = singles.tile([P, P], BF16)
            pa16 = singles.tile([P, B, N], BF16)
            pc_s = singles.tile([P, B * NI], F32)
            strip = singles.tile([P, B, NI], F32)
            acc = singles.tile([P, B], F32)

            # Preload the Relu activation table while the DMA streams in.
            warm = singles.tile([P, 1], F32)
            nc.vector.memset(warm, 0.0)
            nc.scalar.activation(out=warm, in_=warm, func=ACTF.Relu, scale=1.0)

            nc.sync.dma_start(out=blob_s[:, :], in_=blob[:, :])
            nc.sync.dma_start(out=w2bd_s16, in_=w2bd16[:, :])

            o = 0
            xyt_s = blob_s[:, o : o + B * N].rearrange("p (b n) -> p b n", b=B)
            o += B * N
            # xitb: [65, B*NI] — rows 0-63 Xi^T, row 64 = 1.0 (bias lane)
            xitb_s = blob_s[0 : D + 1, o : o + B * NI]
            o += B * NI
            w1ad_s = blob_s[:, o : o + P]
            o += P
            # w1cb: [65, 128] — rows 0-63 = [W1c_xx | W1c_yx], row 64 = b1^T
            w1cb_s = blob_s[0 : D + 1, o : o + P]
            o += P
            assert o == BLOB_W

            # --- setup: pc = [W1c|b1]^T @ [Xi;1] (includes b1) for all
            # (b,i); pa16[b] = bf16(blockdiag(W1a)^T @ [X^T;Y^T][b]) for
            # the Act-bias path.
            pc_ps = wk_pool.tile([P, 2, 512], F32, tag="wk")
            nc.tensor.matmul(
                pc_ps[:, 0, 0 : B * NI], lhsT=w1cb_s, rhs=xitb_s,
                start=True, stop=True,
            )
            nc.scalar.activation(
                out=pc_s, in_=pc_ps[:, 0, 0 : B * NI], func=ACTF.Copy, scale=1.0
            )
            for b0 in range(0, B, 2):
                pa_ps = wk_pool.tile([P, 2, 512], F32, tag="wk")
                for g in range(2):
                    nc.tensor.matmul(
                        pa_ps[:, g, 0:N], lhsT=w1ad_s, rhs=xyt_s[:, b0 + g, :],
                        start=True, stop=True,
                    )
                    nc.vector.tensor_copy(
                        out=pa16[:, b0 + g, :], in_=pa_ps[:, g, 0:N]
                    )

            # --- main loop: 2 i's per work tile, software-pipelined so
            # the PE's in-order queue sees pre(k+1) before W2(k): each
            # pair's relu latency hides under the next pair's pre work.
            def stage1(b, pair):
                ii = 2 * pair
                k0 = b * NI + ii
                wk = wk_pool.tile([P, 2, 512], F32, tag="wk")
                rp2 = rp_pool.tile([P, 2, N], BF16, tag="rp")
                if (pair % ACT_BIAS_MOD) in ACT_BIAS_PAIRS:
                    # relu(pa16 + pc[:,k]) on the Act bias port; the
                    # PE only runs the W2 matmul for these i's.
                    for g in range(2):
                        nc.scalar.activation(
                            out=rp2[:, g, :], in_=pa16[:, b, :],
                            func=ACTF.Relu,
                            bias=pc_s[:, k0 + g : k0 + g + 1], scale=1.0,
                        )
                else:
                    # pre = pa (recompute) + pc-broadcast, then one
                    # batched bias-free relu over the pair.
                    for g in range(2):
                        nc.tensor.matmul(
                            wk[:, g, 0:N], lhsT=w1ad_s, rhs=xyt_s[:, b, :],
                            start=True, stop=False,
                        )
                        nc.tensor.matmul(
                            wk[:, g, 0:N], lhsT=w1cb_s,
                            rhs=xitb_s[:, k0 + g : k0 + g + 1].broadcast_to(
                                [D + 1, N]
                            ),
                            start=False, stop=True,
                        )
                    nc.scalar.activation(
                        out=rp2, in_=wk[:, :, 0:N],
                        func=ACTF.Relu, scale=1.0,
                    )
                return (wk, rp2, b, ii)

            def stage2(st):
                wk, rp2, b, ii = st
                for g in range(2):
                    nc.tensor.matmul(
                        wk[:, g, 0:N], lhsT=w2bd_s16, rhs=rp2[:, g, :],
                        start=True, stop=True,
                    )
                nc.vector.tensor_reduce(
                    out=strip[:, b, ii : ii + 2], in_=wk[:, :, 0:N],
                    axis=AX.X, op=ALU.max,
                )

            pending = None
            for b in range(B):
                for pair in range(NI // 2):
                    st = stage1(b, pair)
                    if pending is not None:
                        stage2(pending)
                    pending = st
            stage2(pending)

            nc.vector.tensor_reduce(
                out=acc, in_=strip, axis=AX.X, op=ALU.add
            )
            nc.sync.dma_start(out=out[:, :], in_=acc[:, :])

    nc.compile()
    return nc


def _prep_inputs(X, Y, W_xx1, W_yx1, b_xx1, b_yx1, W_xx2, W_yx2):
    """Host-side input prep shared by all cores (except xit)."""
    f = np.float32
    XYT = np.ascontiguousarray(
        np.concatenate([X.transpose(0, 2, 1), Y.transpose(0, 2, 1)], axis=1), f
    )  # [B, 128, N]
    W1ad = np.zeros((P, P), f)
    W1ad[:D, :H] = W_xx1[:D]
    W1ad[D:, H:] = W_yx1[:D]
    W1c = np.ascontiguousarray(np.concatenate([W_xx1[D:], W_yx1[D:]], axis=1), f)
    b1v = np.concatenate([b_xx1, b_yx1]).reshape(P, 1).astype(f)
    W2bd = np.zeros((P, P), f)
    W2bd[:H, :H] = W_xx2
    W2bd[H:, H:] = W_yx2
    return XYT, W1ad, W1c, b1v, W2bd


def _pack_blob(XYT, XiT, W1ad, W1c, b1v, W2bd):
    """Pack per-core inputs into the [P, BLOB_W] blob (see build_nc)."""
    f = np.float32
    blob = np.zeros((P, BLOB_W), f)
    o = 0
    blob[:, o : o + B * N] = XYT.transpose(1, 0, 2).reshape(P, B * N)
    o += B * N
    blob[:D, o : o + B * NI] = XiT.transpose(1, 0, 2).reshape(D, B * NI)
    blob[D, o : o + B * NI] = 1.0
    o += B * NI
    blob[:, o : o + P] = W1ad
    o += P
    blob[:D, o : o + P] = W1c
    blob[D, o : o + P] = b1v[:, 0]
    o += P
    assert o == BLOB_W
    return blob


def kernel(
    X, Y,
    W_xx1, b_xx1, W_xx2, b_xx2,
    W_xy1, b_xy1, W_xy2, b_xy2,
    W_yx1, b_yx1, W_yx2, b_yx2,
    W_yy1, b_yy1, W_yy2, b_yy2,
    Wd1, bd1, Wd2, bd2,
    _trace=False, _tmpdir=None,
):
    f = np.float32
    X = np.asarray(X, f)
    Y = np.asarray(Y, f)
    XYT, W1ad, W1c, b1v, W2bd = _prep_inputs(
        X, Y, W_xx1, W_yx1, b_xx1, b_yx1, W_xx2, W_yx2
    )
    import ml_dtypes
    W2bd16 = np.ascontiguousarray(W2bd.astype(ml_dtypes.bfloat16))

    in_maps = []
    for c in range(NCORES):
        XiT = np.ascontiguousarray(
            X[:, c * NI : (c + 1) * NI, :].transpose(0, 2, 1), f
        )  # [B, 64, NI]
        in_maps.append(
            {
                "blob": _pack_blob(XYT, XiT, W1ad, W1c, b1v, W2bd),
                "w2bd16": W2bd16,
            }
        )

    nc = build_nc()
    res = run_bass_kernel_spmd(
        nc,
        in_maps,
        core_ids=list(range(NCORES)),
        trace=_trace,
        tmpdir=_tmpdir,
    )
    acc = np.zeros((P, B), np.float64)
    for r in res.results:
        acc += r["out"].astype(np.float64)
    acc = acc.astype(f)

    # acc[k, b] = sum_i max_j (relu_pre @ W2)[k]  for xx (k<64) / yx (k>=64)
    Zdiff = (acc[:H] - acc[H:]).T + N * (b_xx2 - b_yx2)[None, :]  # [B, H]
    z = np.concatenate([Zdiff, Zdiff], axis=1).astype(f)  # [B, 2H]
    h = np.maximum(z @ Wd1 + bd1, 0.0).astype(f)
    outv = (h @ Wd2 + bd2).astype(f)
    if _trace:
        return outv, res
    return outv
